# revision 21
# baseline (speedup 1.0000x reference)
"""Multi-head attention kernel for 8 Trainium2 NeuronCores.

Problem: B=16, S=512, D=768, H=12 heads (dk=64), fp32.
  y = softmax(QK^T/sqrt(dk) + mask*(-1e9) + adj) V, with QKV/out projections.

Strategy: data-parallel over batch (2 batches per core). On the host we
pre-transpose activations and weights so the device kernel needs zero
on-device transposes; everything on-device is matmul + softmax arithmetic.

Device dataflow (per core, per batch, "transposed domain"):
  QT[e,i]  = (Wq/8)T.T-contracted proj of xqT        (e on partitions)
  KT[e,i]  = proj of xkT
  V[j,e]   = proj of xvT (natural layout: tokens on partitions)
  per head h:
    S.T[j,i]  = KT_h.T-free matmuls (K=dk=64)        -> PSUM
    E.T[j,i]  = exp(S.T + adjT + mask*NEG)           (adj+mask folded on host)
    l[i]      = ones.T @ E.T   (column-sum matmul)   -> PSUM row
    X.T[c,i] += V_h.T-contracted attn@V              (head pair packed in PSUM)
  X.T normalized by 1/l via a tiny broadcast matmul, then output projection
  contracts the head dim back to y[i,e] in natural layout for the store.
"""

import numpy as np

import concourse.bass as bass
from concourse import bacc
import concourse.mybir as mybir
import concourse.tile as tile
from concourse import bass_utils

B, S, D = 16, 512, 768
H, DK = 12, 64
NCORES = 8
BC = B // NCORES  # batches per core
P = 128
DC = D // P  # 6 chunks of d_model
SC = S // P  # 4 chunks of sequence
NEG = np.float32(-1e9)
F32 = mybir.dt.float32
F32R = mybir.dt.float32r
AF = mybir.ActivationFunctionType


def build_program(use_f32r=True):
    nc = bacc.Bacc()
    # fp32r: fp32-width storage the PE consumes at bf16 rate. walrus requires
    # every producer of an fp32r matmul operand to write the fp32r dtype, so
    # DRAM params and SBUF tiles on the matmul paths are declared fp32r
    # (numpy-side both map to float32).
    MM = F32R if use_f32r else F32

    xqT = nc.declare_dram_parameter("xqT", [BC, D, S], MM, isOutput=False)
    xkT = nc.declare_dram_parameter("xkT", [BC, D, S], MM, isOutput=False)
    xvT = nc.declare_dram_parameter("xvT", [BC, D, S], MM, isOutput=False)
    adjT = nc.declare_dram_parameter("adjT", [BC, S, S], F32, isOutput=False)
    WqT = nc.declare_dram_parameter("WqT", [D, D], MM, isOutput=False)
    WkT = nc.declare_dram_parameter("WkT", [D, D], MM, isOutput=False)
    WvT = nc.declare_dram_parameter("WvT", [D, D], MM, isOutput=False)
    WoT = nc.declare_dram_parameter("WoT", [D, D], MM, isOutput=False)
    bqd = nc.declare_dram_parameter("bqd", [D], F32, isOutput=False)
    bkd = nc.declare_dram_parameter("bkd", [D], F32, isOutput=False)
    bvd = nc.declare_dram_parameter("bvd", [D], F32, isOutput=False)
    bod = nc.declare_dram_parameter("bod", [D], F32, isOutput=False)
    y = nc.declare_dram_parameter("y", [BC, S, D], F32, isOutput=True)

    with tile.TileContext(nc) as tc:
        with (
            tc.tile_pool(name="wpool", bufs=1) as wpool,
            tc.tile_pool(name="wqkv", bufs=2) as wqkv,
            tc.tile_pool(name="xpool", bufs=1) as xpool,
            tc.tile_pool(name="qkpool", bufs=3) as qkpool,
            tc.tile_pool(name="vpool", bufs=1) as vpool,
            tc.tile_pool(name="adjpool", bufs=1) as adjpool,
            tc.tile_pool(name="etpool", bufs=2) as etpool,
            tc.tile_pool(name="xopool", bufs=1) as xopool,
            tc.tile_pool(name="lpool", bufs=2) as lpool,
            tc.tile_pool(name="lbpool", bufs=2) as lbpool,
            tc.tile_pool(name="ypool", bufs=2) as ypool,
            tc.tile_pool(name="pp", bufs=2, space="PSUM") as pp,
            tc.tile_pool(name="sp", bufs=2, space="PSUM") as sp,
            tc.tile_pool(name="lp", bufs=1, space="PSUM") as lp,
            tc.tile_pool(name="xp", bufs=2, space="PSUM") as xp,
            tc.tile_pool(name="bp", bufs=1, space="PSUM") as bp,
        ):
            # ---- one-time constants ----
            # per-head layout (head on free dim) so outproj lhsT/rhs both sit
            # at partition base 0 (fp32r matmul requirement)
            wo_sb = wpool.tile([DK, H, D], MM)
            nc.sync.dma_start(wo_sb, WoT.rearrange("(h c) e -> c h e", c=DK))
            bq_sb = wpool.tile([P, DC], F32)
            nc.sync.dma_start(bq_sb, bqd.rearrange("(c p) -> p c", p=P))
            bk_sb = wpool.tile([P, DC], F32)
            nc.sync.dma_start(bk_sb, bkd.rearrange("(c p) -> p c", p=P))
            bvB = wpool.tile([P, D], F32)
            nc.sync.dma_start(bvB, bvd[None, :].to_broadcast((P, D)))
            boB = wpool.tile([P, D], F32)
            nc.sync.dma_start(boB, bod[None, :].to_broadcast((P, D)))
            onesf_sb = wpool.tile([P, 1], F32)
            nc.vector.memset(onesf_sb, 1.0)
            ones_sb = wpool.tile([P, 1], MM)
            nc.vector.tensor_copy(ones_sb, onesf_sb)
            onesrf_sb = wpool.tile([1, DK], F32)
            nc.vector.memset(onesrf_sb, 1.0)
            onesr_sb = wpool.tile([1, DK], MM)
            nc.vector.tensor_copy(onesr_sb, onesrf_sb)

            for b in range(BC):
                # ---- load activations (transposed) and bias matrix ----
                xv_sb = xpool.tile([P, DC, S], MM, tag="xv")
                nc.sync.dma_start(xv_sb, xvT[b].rearrange("(c p) i -> p c i", p=P))
                xq_sb = xpool.tile([P, DC, S], MM, tag="xq")
                nc.sync.dma_start(xq_sb, xqT[b].rearrange("(c p) i -> p c i", p=P))
                xk_sb = xpool.tile([P, DC, S], MM, tag="xk")
                nc.sync.dma_start(xk_sb, xkT[b].rearrange("(c p) i -> p c i", p=P))
                adj_sb = adjpool.tile([P, SC, S], F32, tag="adj")
                nc.sync.dma_start(adj_sb, adjT[b].rearrange("(c p) i -> p c i", p=P))

                # ---- V projection first (natural layout: tokens on partitions);
                # weights stream through a 2-slot pool, reloaded per batch ----
                wv_sb = wqkv.tile([P, DC, D], MM, tag="w", name=f"wv_{b}")
                nc.sync.dma_start(wv_sb, WvT.rearrange("(c p) e -> p c e", p=P))
                v_sb = vpool.tile([P, SC, D], MM, tag="v")
                for sc in range(SC):
                    for hf in range(2):
                        ps_v = pp.tile([P, S], F32, tag="pp", name=f"psv_{b}_{sc}_{hf}")
                        pv = ps_v[:, : D // 2]
                        for dc in range(DC):
                            nc.tensor.matmul(
                                pv,
                                lhsT=xv_sb[:, dc, sc * P : (sc + 1) * P],
                                rhs=wv_sb[:, dc, hf * (D // 2) : (hf + 1) * (D // 2)],
                                start=(dc == 0),
                                stop=(dc == DC - 1),
                            )
                        nc.vector.tensor_add(
                            v_sb[:, sc, hf * (D // 2) : (hf + 1) * (D // 2)],
                            pv,
                            bvB[:, hf * (D // 2) : (hf + 1) * (D // 2)],
                        )

                # ---- Q/K projections (outputs transposed: e on partitions),
                # one [P, S] tile per 128-wide chunk so attention on early
                # chunks overlaps with later projection chunks ----
                wq_sb = wqkv.tile([P, DC, D], MM, tag="w", name=f"wq_{b}")
                nc.sync.dma_start(wq_sb, WqT.rearrange("(c p) e -> p c e", p=P))
                wk_sb = wqkv.tile([P, DC, D], MM, tag="w", name=f"wk_{b}")
                nc.sync.dma_start(wk_sb, WkT.rearrange("(c p) e -> p c e", p=P))
                qts, kts = [], []
                for eb in range(DC):
                    ps_q = pp.tile([P, S], F32, tag="pp", name=f"psq_{b}_{eb}")
                    for dc in range(DC):
                        nc.tensor.matmul(
                            ps_q,
                            lhsT=wq_sb[:, dc, eb * P : (eb + 1) * P],
                            rhs=xq_sb[:, dc, :],
                            start=(dc == 0),
                            stop=(dc == DC - 1),
                        )
                    qt_c = qkpool.tile([P, S], MM, tag="qt", name=f"qt_{b}_{eb}")
                    nc.scalar.activation(
                        qt_c, ps_q, AF.Identity, bias=bq_sb[:, eb : eb + 1]
                    )
                    qts.append(qt_c)
                    ps_k = pp.tile([P, S], F32, tag="pp", name=f"psk_{b}_{eb}")
                    for dc in range(DC):
                        nc.tensor.matmul(
                            ps_k,
                            lhsT=wk_sb[:, dc, eb * P : (eb + 1) * P],
                            rhs=xk_sb[:, dc, :],
                            start=(dc == 0),
                            stop=(dc == DC - 1),
                        )
                    kt_c = qkpool.tile([P, S], MM, tag="kt", name=f"kt_{b}_{eb}")
                    nc.scalar.activation(
                        kt_c, ps_k, AF.Identity, bias=bk_sb[:, eb : eb + 1]
                    )
                    kts.append(kt_c)

                # ---- attention ----
                # fp32r matmuls must write PSUM at partition base 0, so every
                # head gets its own [64/1, S] psum tiles and xout keeps heads
                # side by side on 64 partitions; the output projection then
                # contracts per head with K=64.
                xout_sb = xopool.tile([DK, H, S], MM, tag="xout")
                for h in range(H):
                    po = (h % 2) * DK
                    ch = h // 2
                    et = etpool.tile([P, SC, S], MM, tag="et", name=f"et_{b}_{h}")
                    lps = lp.tile([1, S], F32, tag="l", name=f"lps_{b}_{h}")
                    for jc in range(SC):
                        ps_s = sp.tile([P, S], F32, tag="s", name=f"pss_{b}_{h}_{jc}")
                        nc.tensor.matmul(
                            ps_s,
                            lhsT=kts[ch][po : po + DK, jc * P : (jc + 1) * P],
                            rhs=qts[ch][po : po + DK, :],
                            start=True,
                            stop=True,
                        )
                        nc.vector.tensor_add(ps_s, ps_s, adj_sb[:, jc, :])
                        nc.scalar.activation(et[:, jc, :], ps_s, AF.Exp)
                        nc.tensor.matmul(
                            lps,
                            lhsT=ones_sb,
                            rhs=et[:, jc, :],
                            start=(jc == 0),
                            stop=(jc == SC - 1),
                        )
                    linv_sb = lpool.tile([1, S], MM, tag="linv", name=f"linv_{b}_{h}")
                    with nc.allow_low_precision(reason="1/l softmax scale in fp32r"):
                        nc.vector.reciprocal(linv_sb, lps)
                    xps = xp.tile([DK, S], F32, tag="x", name=f"xps_{b}_{h}")
                    for jc in range(SC):
                        nc.tensor.matmul(
                            xps,
                            lhsT=v_sb[:, jc, h * DK : (h + 1) * DK],
                            rhs=et[:, jc, :],
                            start=(jc == 0),
                            stop=(jc == SC - 1),
                        )
                    # broadcast 1/l over the head's 64 partitions via a K=1 matmul
                    bps = bp.tile([DK, S], F32, tag="b", name=f"bps_{b}_{h}")
                    nc.tensor.matmul(bps, lhsT=onesr_sb, rhs=linv_sb, start=True, stop=True)
                    linvb_sb = lbpool.tile([DK, S], F32, tag="linvb", name=f"linvb_{b}_{h}")
                    nc.scalar.copy(linvb_sb, bps)
                    nc.vector.tensor_mul(xout_sb[:, h, :], xps, linvb_sb)

                # ---- output projection (back to natural layout) ----
                for ib in range(SC):
                    y_sb = ypool.tile([P, D], F32, tag="y", name=f"y_{b}_{ib}")
                    for hf in range(2):
                        ps_y = pp.tile([P, S], F32, tag="pp", name=f"psy_{b}_{ib}_{hf}")
                        py = ps_y[:, : D // 2]
                        for h in range(H):
                            nc.tensor.matmul(
                                py,
                                lhsT=xout_sb[:, h, ib * P : (ib + 1) * P],
                                rhs=wo_sb[:, h, hf * (D // 2) : (hf + 1) * (D // 2)],
                                start=(h == 0),
                                stop=(h == H - 1),
                            )
                        nc.vector.tensor_add(
                            y_sb[:, hf * (D // 2) : (hf + 1) * (D // 2)],
                            py,
                            boB[:, hf * (D // 2) : (hf + 1) * (D // 2)],
                        )
                    nc.sync.dma_start(y[b, ib * P : (ib + 1) * P, :], y_sb)

    nc.finalize()
    return nc


def host_prep(q, k, v, mask, adj, Wq, bq, Wk, bk, Wv, bv, Wo, bo):
    """Build per-core input maps (numpy layout prep; no math beyond adds/scales)."""
    f = np.float32
    q = np.asarray(q, f)
    k = np.asarray(k, f)
    v = np.asarray(v, f)
    mask = np.asarray(mask, f).reshape(B, S)
    adj = np.asarray(adj, f).reshape(B, S, S)
    scale = f(1.0) / np.sqrt(f(DK))

    WqTs = np.ascontiguousarray(np.asarray(Wq, f).T * scale)
    WkT = np.ascontiguousarray(np.asarray(Wk, f).T)
    WvT = np.ascontiguousarray(np.asarray(Wv, f).T)
    WoT = np.ascontiguousarray(np.asarray(Wo, f).T)
    bqs = np.asarray(bq, f) * scale
    bk_ = np.asarray(bk, f)
    bv_ = np.asarray(bv, f)
    bo_ = np.asarray(bo, f)

    # scores bias, transposed: adjT[b][j,i] = adj[b][i,j] + NEG*mask[b][j]
    adjT = np.ascontiguousarray(adj.transpose(0, 2, 1) + (NEG * mask)[:, :, None])

    qT = np.ascontiguousarray(q.transpose(0, 2, 1))
    kT = np.ascontiguousarray(k.transpose(0, 2, 1))
    vT = np.ascontiguousarray(v.transpose(0, 2, 1))

    in_maps = []
    for c in range(NCORES):
        sl = slice(c * BC, (c + 1) * BC)
        in_maps.append(
            {
                "xqT": qT[sl],
                "xkT": kT[sl],
                "xvT": vT[sl],
                "adjT": adjT[sl],
                "WqT": WqTs,
                "WkT": WkT,
                "WvT": WvT,
                "WoT": WoT,
                "bqd": bqs,
                "bkd": bk_,
                "bvd": bv_,
                "bod": bo_,
            }
        )
    return in_maps


_PROGRAM = None


def _get_program():
    global _PROGRAM
    if _PROGRAM is None:
        _PROGRAM = build_program()
    return _PROGRAM


def kernel(q, k, v, mask, adj, Wq, bq, Wk, bk, Wv, bv, Wo, bo):
    nc = _get_program()
    in_maps = host_prep(q, k, v, mask, adj, Wq, bq, Wk, bk, Wv, bv, Wo, bo)
    res = bass_utils.run_bass_kernel_spmd(nc, in_maps, list(range(NCORES)))
    out = np.concatenate([np.asarray(res.results[i]["y"]) for i in range(NCORES)], axis=0)
    return out.astype(np.float32)


# revision 26
# speedup vs baseline: 1.4042x; 1.4042x over previous
"""Multi-head attention kernel for 8 Trainium2 NeuronCores.

Problem: B=16, S=512, D=768, H=12 heads (dk=64), fp32.
  y = softmax(QK^T/sqrt(dk) + mask*(-1e9) + adj) V, with QKV/out projections.

Strategy: data-parallel over batch (2 batches per core). On the host we
pre-transpose activations and weights so the device kernel needs zero
on-device transposes; everything on-device is matmul + softmax arithmetic.

Device dataflow (per core, per batch, "transposed domain"):
  QT[e,i]  = (Wq/8)T-contracted proj of xqT          (e on partitions)
  KT[e,i]  = proj of xkT
  V'[j,e'] = proj of xvT with Wv augmented on the host by one zero column +
             bias 1.0 per head, so each head carries a built-in ones column
             (natural layout: tokens on partitions, e' = h*65 + c)
  per head h:
    S.T[j,i]  = KT_h matmuls (K=dk=64)               -> PSUM
    E.T[j,i]  = exp(S.T + adjT + mask*NEG)           (adj+mask folded on host)
    X'[c,i]  += V'_h attn@V; row 64 = softmax denom l[i]  (M=65)
  l broadcast to 64 partitions by a K=1 matmul, reciprocal on 64 lanes,
  normalize during PSUM copyback; odd heads DMA-packed to partitions 64:128
  so the output projection contracts head pairs with K=128 back to y[i,e].
"""

import numpy as np

import concourse.bass as bass
from concourse import bacc
import concourse.mybir as mybir
import concourse.tile as tile
from concourse import bass_utils

B, S, D = 16, 512, 768
H, DK = 12, 64
DKE = DK + 1  # head width incl. the ones column in the augmented V
VE = H * DKE  # 780
NCORES = 8
BC = B // NCORES  # batches per core
P = 128
DC = D // P  # 6 chunks of d_model
SC = S // P  # 4 chunks of sequence
NEG = np.float32(-1e9)
F32 = mybir.dt.float32
F32R = mybir.dt.float32r
AF = mybir.ActivationFunctionType


def build_program(use_f32r=True):
    nc = bacc.Bacc()
    # fp32r: fp32-width storage the PE consumes at bf16 rate. walrus requires
    # every producer of an fp32r matmul operand to write the fp32r dtype, so
    # DRAM params and SBUF tiles on the matmul paths are declared fp32r
    # (numpy-side both map to float32).
    MM = F32R if use_f32r else F32

    xqT = nc.declare_dram_parameter("xqT", [BC, D, S], MM, isOutput=False)
    xkT = nc.declare_dram_parameter("xkT", [BC, D, S], MM, isOutput=False)
    xvT = nc.declare_dram_parameter("xvT", [BC, D, S], MM, isOutput=False)
    adjT = nc.declare_dram_parameter("adjT", [BC, S, S], F32, isOutput=False)
    WqT = nc.declare_dram_parameter("WqT", [D, D], MM, isOutput=False)
    WkT = nc.declare_dram_parameter("WkT", [D, D], MM, isOutput=False)
    WvT = nc.declare_dram_parameter("WvT", [D, VE], MM, isOutput=False)
    WoT = nc.declare_dram_parameter("WoT", [D, D], MM, isOutput=False)
    bqd = nc.declare_dram_parameter("bqd", [D], F32, isOutput=False)
    bkd = nc.declare_dram_parameter("bkd", [D], F32, isOutput=False)
    bvd = nc.declare_dram_parameter("bvd", [VE], F32, isOutput=False)
    bod = nc.declare_dram_parameter("bod", [D], F32, isOutput=False)
    y = nc.declare_dram_parameter("y", [BC, S, D], F32, isOutput=True)

    with tile.TileContext(nc) as tc:
        with (
            tc.tile_pool(name="wpool", bufs=1) as wpool,
            tc.tile_pool(name="xpool", bufs=1) as xpool,
            tc.tile_pool(name="qkpool", bufs=3) as qkpool,
            tc.tile_pool(name="vpool", bufs=1) as vpool,
            tc.tile_pool(name="adjpool", bufs=1) as adjpool,
            tc.tile_pool(name="etpool", bufs=2) as etpool,
            tc.tile_pool(name="xopool", bufs=1) as xopool,
            tc.tile_pool(name="lpool", bufs=2) as lpool,
            tc.tile_pool(name="lbpool", bufs=2) as lbpool,
            tc.tile_pool(name="tmpool", bufs=2) as tmpool,
            tc.tile_pool(name="ypool", bufs=2) as ypool,
            tc.tile_pool(name="pp", bufs=2, space="PSUM") as pp,
            tc.tile_pool(name="sp", bufs=3, space="PSUM") as sp,
            tc.tile_pool(name="xp", bufs=2, space="PSUM") as xp,
            tc.tile_pool(name="bp", bufs=1, space="PSUM") as bp,
        ):
            # ---- one-time constants ----
            wq_sb = wpool.tile([P, DC, D], MM)
            nc.sync.dma_start(wq_sb, WqT.rearrange("(c p) e -> p c e", p=P))
            wk_sb = wpool.tile([P, DC, D], MM)
            nc.sync.dma_start(wk_sb, WkT.rearrange("(c p) e -> p c e", p=P))
            wv_sb = wpool.tile([P, DC, VE], MM)
            nc.sync.dma_start(wv_sb, WvT.rearrange("(c p) e -> p c e", p=P))
            wo_sb = wpool.tile([P, DC, D], MM)
            nc.sync.dma_start(wo_sb, WoT.rearrange("(c p) e -> p c e", p=P))
            bq_sb = wpool.tile([P, DC], F32)
            nc.sync.dma_start(bq_sb, bqd.rearrange("(c p) -> p c", p=P))
            bk_sb = wpool.tile([P, DC], F32)
            nc.sync.dma_start(bk_sb, bkd.rearrange("(c p) -> p c", p=P))
            bvB = wpool.tile([P, VE], F32)
            nc.sync.dma_start(bvB, bvd[None, :].to_broadcast((P, VE)))
            boB = wpool.tile([P, D], F32)
            nc.sync.dma_start(boB, bod[None, :].to_broadcast((P, D)))
            # row 64 of a [65, DK] ones tile: lhsT for the K=1 broadcast of
            # the softmax denominator (matmul operand bases must match: the
            # denominator lives on partition 64 of the attn@V psum)
            ones64f_sb = wpool.tile([DKE, DK], F32)
            nc.vector.memset(ones64f_sb[DK : DK + 1, :], 1.0)
            ones64_sb = wpool.tile([DKE, DK], MM)
            nc.vector.tensor_copy(ones64_sb[DK : DK + 1, :], ones64f_sb[DK : DK + 1, :])

            for b in range(BC):
                # ---- load activations (transposed) and bias matrix ----
                xv_sb = xpool.tile([P, DC, S], MM, tag="xv")
                nc.sync.dma_start(xv_sb, xvT[b].rearrange("(c p) i -> p c i", p=P))
                xq_sb = xpool.tile([P, DC, S], MM, tag="xq")
                nc.sync.dma_start(xq_sb, xqT[b].rearrange("(c p) i -> p c i", p=P))
                xk_sb = xpool.tile([P, DC, S], MM, tag="xk")
                nc.sync.dma_start(xk_sb, xkT[b].rearrange("(c p) i -> p c i", p=P))
                adj_sb = adjpool.tile([P, SC, S], F32, tag="adj")
                nc.sync.dma_start(adj_sb, adjT[b].rearrange("(c p) i -> p c i", p=P))

                # ---- V projection (natural layout: tokens on partitions,
                # e' = h*65+c with a built-in ones column per head) ----
                v_sb = vpool.tile([P, SC, VE], MM, tag="v")
                for sc in range(SC):
                    for hf in range(2):
                        ps_v = pp.tile([P, S], F32, tag="pp", name=f"psv_{b}_{sc}_{hf}")
                        pv = ps_v[:, : VE // 2]
                        for dc in range(DC):
                            nc.tensor.matmul(
                                pv,
                                lhsT=xv_sb[:, dc, sc * P : (sc + 1) * P],
                                rhs=wv_sb[:, dc, hf * (VE // 2) : (hf + 1) * (VE // 2)],
                                start=(dc == 0),
                                stop=(dc == DC - 1),
                            )
                        nc.vector.tensor_add(
                            v_sb[:, sc, hf * (VE // 2) : (hf + 1) * (VE // 2)],
                            pv,
                            bvB[:, hf * (VE // 2) : (hf + 1) * (VE // 2)],
                        )

                # ---- Q/K projections (outputs transposed: e on partitions),
                # one [P, S] tile per 128-wide chunk so attention on early
                # chunks overlaps with later projection chunks ----
                qts, kts = [], []
                for eb in range(DC):
                    ps_q = pp.tile([P, S], F32, tag="pp", name=f"psq_{b}_{eb}")
                    for dc in range(DC):
                        nc.tensor.matmul(
                            ps_q,
                            lhsT=wq_sb[:, dc, eb * P : (eb + 1) * P],
                            rhs=xq_sb[:, dc, :],
                            start=(dc == 0),
                            stop=(dc == DC - 1),
                        )
                    qt_c = qkpool.tile([P, S], MM, tag="qt", name=f"qt_{b}_{eb}")
                    nc.scalar.activation(
                        qt_c, ps_q, AF.Identity, bias=bq_sb[:, eb : eb + 1]
                    )
                    qts.append(qt_c)
                    ps_k = pp.tile([P, S], F32, tag="pp", name=f"psk_{b}_{eb}")
                    for dc in range(DC):
                        nc.tensor.matmul(
                            ps_k,
                            lhsT=wk_sb[:, dc, eb * P : (eb + 1) * P],
                            rhs=xk_sb[:, dc, :],
                            start=(dc == 0),
                            stop=(dc == DC - 1),
                        )
                    kt_c = qkpool.tile([P, S], MM, tag="kt", name=f"kt_{b}_{eb}")
                    nc.scalar.activation(
                        kt_c, ps_k, AF.Identity, bias=bk_sb[:, eb : eb + 1]
                    )
                    kts.append(kt_c)

                # ---- attention ----
                # fp32r matmuls must write PSUM at partition base 0; head
                # pairs are packed onto 128 partitions with a lane-crossing
                # DMA for the odd head so the output projection runs K=128.
                xout_sb = xopool.tile([P, DC, S], MM, tag="xout")
                for h in range(H):
                    po = (h % 2) * DK
                    ch = h // 2
                    et = etpool.tile([P, SC, S], MM, tag="et", name=f"et_{b}_{h}")
                    for jc in range(SC):
                        ps_s = sp.tile([P, S], F32, tag="s", name=f"pss_{b}_{h}_{jc}")
                        nc.tensor.matmul(
                            ps_s,
                            lhsT=kts[ch][po : po + DK, jc * P : (jc + 1) * P],
                            rhs=qts[ch][po : po + DK, :],
                            start=True,
                            stop=True,
                        )
                        nc.vector.tensor_add(ps_s, ps_s, adj_sb[:, jc, :])
                        nc.scalar.activation(et[:, jc, :], ps_s, AF.Exp)
                    xps = xp.tile([DKE, S], F32, tag="x", name=f"xps_{b}_{h}")
                    for jc in range(SC):
                        nc.tensor.matmul(
                            xps,
                            lhsT=v_sb[:, jc, h * DKE : (h + 1) * DKE],
                            rhs=et[:, jc, :],
                            start=(jc == 0),
                            stop=(jc == SC - 1),
                        )
                    # row 64 of xps is l = sum_j E.T; broadcast it over the
                    # head's 64 partitions (K=1 matmul), then 1/x on 64 lanes
                    l_sb = lpool.tile([DKE, S], MM, tag="l", name=f"l_{b}_{h}")
                    nc.scalar.copy(l_sb[DK : DK + 1, :], xps[DK : DK + 1, :])
                    bps = bp.tile([DK, S], F32, tag="b", name=f"bps_{b}_{h}")
                    nc.tensor.matmul(
                        bps,
                        lhsT=ones64_sb[DK : DK + 1, :],
                        rhs=l_sb[DK : DK + 1, :],
                        start=True,
                        stop=True,
                    )
                    linvb_sb = lbpool.tile([DK, S], F32, tag="linvb", name=f"linvb_{b}_{h}")
                    nc.vector.reciprocal(linvb_sb, bps)
                    if h % 2 == 0:
                        nc.vector.tensor_mul(
                            xout_sb[0:DK, h // 2, :], xps[0:DK, :], linvb_sb
                        )
                    else:
                        tmp_sb = tmpool.tile([DK, S], MM, tag="tmp", name=f"tmp_{b}_{h}")
                        nc.vector.tensor_mul(tmp_sb, xps[0:DK, :], linvb_sb)
                        nc.sync.dma_start(xout_sb[DK:P, h // 2, :], tmp_sb)

                # ---- output projection (back to natural layout) ----
                for ib in range(SC):
                    y_sb = ypool.tile([P, D], F32, tag="y", name=f"y_{b}_{ib}")
                    for hf in range(2):
                        ps_y = pp.tile([P, S], F32, tag="pp", name=f"psy_{b}_{ib}_{hf}")
                        py = ps_y[:, : D // 2]
                        for fc in range(DC):
                            nc.tensor.matmul(
                                py,
                                lhsT=xout_sb[:, fc, ib * P : (ib + 1) * P],
                                rhs=wo_sb[:, fc, hf * (D // 2) : (hf + 1) * (D // 2)],
                                start=(fc == 0),
                                stop=(fc == DC - 1),
                            )
                        nc.vector.tensor_add(
                            y_sb[:, hf * (D // 2) : (hf + 1) * (D // 2)],
                            py,
                            boB[:, hf * (D // 2) : (hf + 1) * (D // 2)],
                        )
                    nc.sync.dma_start(y[b, ib * P : (ib + 1) * P, :], y_sb)

    nc.finalize()
    return nc


def host_prep(q, k, v, mask, adj, Wq, bq, Wk, bk, Wv, bv, Wo, bo):
    """Build per-core input maps (numpy layout prep; no math beyond adds/scales)."""
    f = np.float32
    q = np.asarray(q, f)
    k = np.asarray(k, f)
    v = np.asarray(v, f)
    mask = np.asarray(mask, f).reshape(B, S)
    adj = np.asarray(adj, f).reshape(B, S, S)
    scale = f(1.0) / np.sqrt(f(DK))

    WqTs = np.ascontiguousarray(np.asarray(Wq, f).T * scale)
    WkT = np.ascontiguousarray(np.asarray(Wk, f).T)
    WoT = np.ascontiguousarray(np.asarray(Wo, f).T)
    bqs = np.asarray(bq, f) * scale
    bk_ = np.asarray(bk, f)
    bo_ = np.asarray(bo, f)
    # augment Wv/bv with a zero column / 1.0 bias at e' = h*65+64 per head,
    # so the V projection emits a ones column that attn@V turns into the
    # softmax denominator
    WvT = np.zeros((D, VE), f)
    bv_ = np.zeros((VE,), f)
    WvT_nat = np.asarray(Wv, f).T
    bv_nat = np.asarray(bv, f)
    for h in range(H):
        WvT[:, h * DKE : h * DKE + DK] = WvT_nat[:, h * DK : (h + 1) * DK]
        bv_[h * DKE : h * DKE + DK] = bv_nat[h * DK : (h + 1) * DK]
        bv_[h * DKE + DK] = 1.0

    # scores bias, transposed: adjT[b][j,i] = adj[b][i,j] + NEG*mask[b][j]
    adjT = np.ascontiguousarray(adj.transpose(0, 2, 1) + (NEG * mask)[:, :, None])

    qT = np.ascontiguousarray(q.transpose(0, 2, 1))
    kT = np.ascontiguousarray(k.transpose(0, 2, 1))
    vT = np.ascontiguousarray(v.transpose(0, 2, 1))

    in_maps = []
    for c in range(NCORES):
        sl = slice(c * BC, (c + 1) * BC)
        in_maps.append(
            {
                "xqT": qT[sl],
                "xkT": kT[sl],
                "xvT": vT[sl],
                "adjT": adjT[sl],
                "WqT": WqTs,
                "WkT": WkT,
                "WvT": WvT,
                "WoT": WoT,
                "bqd": bqs,
                "bkd": bk_,
                "bvd": bv_,
                "bod": bo_,
            }
        )
    return in_maps


_PROGRAM = None


def _get_program():
    global _PROGRAM
    if _PROGRAM is None:
        _PROGRAM = build_program()
    return _PROGRAM


def kernel(q, k, v, mask, adj, Wq, bq, Wk, bk, Wv, bv, Wo, bo):
    nc = _get_program()
    in_maps = host_prep(q, k, v, mask, adj, Wq, bq, Wk, bk, Wv, bv, Wo, bo)
    res = bass_utils.run_bass_kernel_spmd(nc, in_maps, list(range(NCORES)))
    out = np.concatenate([np.asarray(res.results[i]["y"]) for i in range(NCORES)], axis=0)
    return out.astype(np.float32)
